# revision 9
# baseline (speedup 1.0000x reference)
"""RBF kernel regression (Gauss transform) on 8 Trainium2 NeuronCores.

Computes out = K @ alpha where K[b, n] = exp(-||z_b - x_n||^2 / 2),
z: [2048, 64], dataset: [100000, 64], alpha: [100000, 16].

Strategy (sharding_hint): shard dataset/alpha row-wise (N) across 8 cores.
Factorize K = exp(z.x) * exp(-x^2/2) * exp(-z^2/2): fold exp(-x^2/2) into
alpha on the host, apply exp(-z^2/2) on the host at the end. Each core then
computes partial[f, b] = sum_n alpha'[n, f] * exp(z.x_n) over its shard.

The device kernel is ScalarE(exp)-bound: 25.7M exps/core at 1.2GHz/128
lanes is a ~167us floor, so the whole design minimizes per-ACT-instruction
overhead (~350ns each: 222-cycle SBUF access bubble + sem/dispatch):
  - exp runs on [128, 2048] / [128, 1536] PSUM tiles (4+3 banks,
    alternating as the two pipeline slots; 8th bank = accumulator), i.e.
    112 ACTIVATEs instead of 196.
  - per b-quarter pass (512 cols), 98 n-tiles stream through the slots as
    "units" alternating top/bottom PE-row halves so paired K=64 cross
    matmuls overlap on the PE array; alpha-weighted acc matmuls land in
    one merged PSUM bank (rows 0:16 / 32:48 via tile_position col 0/32).
  - outputs DMA straight from PSUM to DRAM (no SBUF staging copy).
  - a dummy exp at kernel start pulls the ~1.5us ACT table load off the
    critical chain; first DMAs are split small so the first cross matmul
    starts ASAP.
"""

import sys

if "/opt/trn_rl_repo" not in sys.path:
    sys.path.insert(0, "/opt/trn_rl_repo")

import numpy as np

B = 2048  # batch (queries)
D = 64  # feature dim
F = 16  # output dim
NCORES = 8
N_FULL = 100000
NS = N_FULL // NCORES  # 12500 rows per core
NT = 98  # n-tiles of 128 rows (12544 padded)
NTH = NT // 2  # 49 tiles per partition-half
NSP = NT * 128  # 12544
HALF_COLS = NTH * 128  # 6272
BCH = 512  # b chunk per pass (PSUM acc bank width)
SLOT_UNITS = (4, 3)  # units per alternating PSUM x-slot (4+3 banks)


def _unit_tile(u, nth=NTH):
    """Unit u (0..2*nth-1) -> (dst col-tile index, top-half?)."""
    return u // 2, (u % 2 == 0)


def _pack_core_inputs(z, dataset, alpha):
    """Host-side packing: returns (in_maps, w) where w[b] = exp(-0.5*||z_b||^2)."""
    import ml_dtypes

    z = np.ascontiguousarray(z, dtype=np.float32)
    dataset = np.ascontiguousarray(dataset, dtype=np.float32)
    alpha = np.ascontiguousarray(alpha, dtype=np.float32)

    zT = z.T  # [64, B]
    zt_packed = np.concatenate([zT, zT], axis=0).astype(np.float16)  # [128, B]
    z_sq = np.sum(z.astype(np.float64) ** 2, axis=1)
    w = np.exp(-0.5 * z_sq)  # [B], applied on host at the end

    in_maps = []
    for c in range(NCORES):
        ds_c = dataset[c * NS : (c + 1) * NS]
        al_c = alpha[c * NS : (c + 1) * NS]
        dsp = np.zeros((NSP, D), np.float32)
        dsp[:NS] = ds_c
        alp = np.zeros((NSP, F), np.float32)
        alp[:NS] = al_c
        # fold exp(-x^2/2) into alpha (float64 to keep tiny magnitudes exact)
        xsq = np.sum(dsp.astype(np.float64) ** 2, axis=1)
        alp = (alp.astype(np.float64) * np.exp(-0.5 * xsq)[:, None]).astype(
            np.float32
        )

        dsT = dsp.T  # [64, NSP]
        dst_packed = np.concatenate(
            [dsT[:, :HALF_COLS], dsT[:, HALF_COLS:]], axis=0
        ).astype(np.float16)  # [128, 6272]: tile t rows 0:64, tile NTH+t rows 64:128

        # alpha in device "unit" order: unit u covers tile u//2 (u even,
        # top half) or NTH + u//2 (u odd, bottom half)
        a3 = alp.reshape(NT, 128, F)  # [NT, 128, F]
        order = [u // 2 if u % 2 == 0 else NTH + u // 2 for u in range(NT)]
        alp_packed = np.ascontiguousarray(
            a3[order].transpose(1, 0, 2).reshape(128, NT * F)
        ).astype(ml_dtypes.bfloat16)

        in_maps.append(
            {
                "zt": np.ascontiguousarray(zt_packed),
                "dst": np.ascontiguousarray(dst_packed),
                "alp": alp_packed,
            }
        )
    return in_maps, w


def build_nc(nt=NT):
    """Build the Bass module. nt can be reduced (multiple of 14) for smoke tests."""
    import concourse.bass as bass
    import concourse.tile as tile
    from concourse import bacc, mybir

    assert nt % 14 == 0
    nth = nt // 2
    half_cols = nth * 128

    f32 = mybir.dt.float32
    f16 = mybir.dt.float16
    bf16 = mybir.dt.bfloat16

    nc = bacc.Bacc("TRN2", target_bir_lowering=False, debug=False)
    zt_d = nc.dram_tensor("zt", [128, B], f16, kind="ExternalInput").ap()
    dst_d = nc.dram_tensor("dst", [128, half_cols], f16, kind="ExternalInput").ap()
    alp_d = nc.dram_tensor("alp", [128, nt * F], bf16, kind="ExternalInput").ap()
    out_d = nc.dram_tensor("out", [64, B], f32, kind="ExternalOutput").ap()

    with tile.TileContext(nc) as tc:
        with (
            tc.tile_pool(name="consts", bufs=1) as consts,
            tc.tile_pool(name="g", bufs=2) as gpool,
            tc.tile_pool(name="ps_x", bufs=1, space="PSUM") as ps_x,
        ):
            warm = consts.tile([128, 8], f32, tag="warm", name="warm")
            warm_o = consts.tile([128, 8], bf16, tag="warmo", name="warmo")
            out_sb = consts.tile([64, B], f32, tag="out", name="out_sb")
            zt_sb = consts.tile([128, B], f16, tag="zt", name="zt")
            dst_sb = consts.tile([128, half_cols], f16, tag="dst", name="dst")
            alp_sb = consts.tile([128, nt * F], bf16, tag="alp", name="alp")

            # First-needed pieces first, on the two fast HWDGE queues
            # (sync + scalar). The slow SWDGE gpsimd queue only carries
            # alpha, which isn't needed until the first acc matmul.
            nc.sync.dma_start(out=zt_sb[:, 0:512], in_=zt_d[:, 0:512])
            nc.scalar.dma_start(out=dst_sb[:, 0:256], in_=dst_d[:, 0:256])
            nc.sync.dma_start(out=dst_sb[:, 256:2048], in_=dst_d[:, 256:2048])
            nc.gpsimd.dma_start(out=alp_sb, in_=alp_d)
            nc.sync.dma_start(out=zt_sb[:, 512:B], in_=zt_d[:, 512:B])
            nc.sync.dma_start(
                out=dst_sb[:, 2048:half_cols], in_=dst_d[:, 2048:half_cols]
            )

            # Warm the Exp activation table while the input DMAs fly so
            # the ~1.5us ACT_TABLE_LOAD stays off the critical ACT chain.
            nc.scalar.memzero(warm)
            nc.scalar.activation(
                out=warm_o, in_=warm, func=mybir.ActivationFunctionType.Exp
            )

            # Flat slot list across all 4 b-passes; software-pipelined
            # emission (cross(s), ACT(s-1), acc(s-2)) so the PE queue
            # interleaves next-slot fills ahead of prior-slot drains and
            # the ACT chain never waits on the PE. Each pass opens with a
            # small 2-unit slot (earlier first ACT), then 4-unit slots.
            slots = []
            si = 0
            for bq in range(4):
                u = 0
                while u < nt:
                    nu = 2 if u == 0 else 4
                    slots.append((bq, si % 2, u, nu))
                    u += nu
                    si += 1

            x_t = {}
            g_t = {}

            def emit_cross(s):
                bq, si, u0, nu = slots[s]
                bs = bq * BCH
                x = ps_x.tile([128, nu * BCH], f32, tag=f"x{si}", name=f"x{si}")
                x_t[s] = x
                for j in range(nu):
                    k, top = _unit_tile(u0 + j, nth)
                    rlo = 0 if top else 64
                    nc.tensor.matmul(
                        x[:, j * BCH : (j + 1) * BCH],
                        lhsT=dst_sb[rlo : rlo + 64, k * 128 : (k + 1) * 128],
                        rhs=zt_sb[rlo : rlo + 64, bs : bs + BCH],
                        start=True,
                        stop=True,
                    )

            def emit_act(s):
                _, si, _, nu = slots[s]
                g = gpool.tile([128, nu * BCH], bf16, tag=f"g{si}", name=f"g{si}")
                g_t[s] = g
                nc.scalar.activation(
                    out=g, in_=x_t[s], func=mybir.ActivationFunctionType.Exp
                )

            def emit_acc(s):
                bq, si, u0, nu = slots[s]
                bs = bq * BCH
                g = g_t.pop(s)
                x = x_t.pop(s)
                # per-slot partial sums accumulate into the LAST 512-col
                # block of this x slot (rows 0:16 top / 32:48 bottom) —
                # ACT already consumed it, and it's the last region the
                # next-next cross fill rewrites, maximizing slack.
                ac = slice((nu - 1) * BCH, nu * BCH)
                tops = [j for j in range(nu) if (u0 + j) % 2 == 0]
                bots = [j for j in range(nu) if (u0 + j) % 2 == 1]
                for grp in (tops, bots):
                    for j in grp:
                        uu = u0 + j
                        _, top = _unit_tile(uu, nth)
                        rows = slice(0, F) if top else slice(32, 32 + F)
                        nc.tensor.matmul(
                            x[rows, ac],
                            lhsT=alp_sb[:, uu * F : (uu + 1) * F],
                            rhs=g[:, j * BCH : (j + 1) * BCH],
                            start=(j == grp[0]),
                            stop=(j == grp[-1]),
                            tile_position=(0, 0) if top else (0, 32),
                        )
                # fold the slot partial into the SBUF accumulator (DVE)
                if u0 == 0:
                    nc.vector.tensor_copy(
                        out=out_sb[0 : 32 + F, bs : bs + BCH],
                        in_=x[0 : 32 + F, ac],
                    )
                else:
                    nc.vector.tensor_add(
                        out_sb[0 : 32 + F, bs : bs + BCH],
                        out_sb[0 : 32 + F, bs : bs + BCH],
                        x[0 : 32 + F, ac],
                    )
                if u0 + nu == nt:
                    nc.sync.dma_start(
                        out=out_d[0 : 32 + F, bs : bs + BCH],
                        in_=out_sb[0 : 32 + F, bs : bs + BCH],
                    )

            for s in range(len(slots) + 2):
                if s < len(slots):
                    emit_cross(s)
                if 0 <= s - 1 < len(slots):
                    emit_act(s - 1)
                if 0 <= s - 2 < len(slots):
                    emit_acc(s - 2)

    nc.compile()
    return nc


_NC_CACHE = []


def run_on_cores(in_maps, trace=False, **kwargs):
    from concourse.bass_utils import run_bass_kernel_spmd

    if not _NC_CACHE:
        _NC_CACHE.append(build_nc())
    return run_bass_kernel_spmd(
        _NC_CACHE[0], in_maps, core_ids=list(range(NCORES)), trace=trace, **kwargs
    )


def kernel(z, dataset, alpha):
    in_maps, w = _pack_core_inputs(z, dataset, alpha)
    res = run_on_cores(in_maps, trace=False)
    total = np.zeros((F, B), np.float64)
    for r in res.results:
        o = r["out"].astype(np.float64)  # [64, B]
        total += o[0:F] + o[32 : 32 + F]
    total *= w[None, :]
    return np.ascontiguousarray(total.T.astype(np.float32))


# revision 10
# speedup vs baseline: 1.4923x; 1.4923x over previous
"""RBF kernel regression (Gauss transform) on 8 Trainium2 NeuronCores.

Computes out = K @ alpha where K[b, n] = exp(-||z_b - x_n||^2 / 2),
z: [2048, 64], dataset: [100000, 64], alpha: [100000, 16].

Strategy (sharding_hint): shard dataset/alpha row-wise (N) across 8 cores.
Factorize K = exp(z.x) * exp(-x^2/2) * exp(-z^2/2): fold exp(-x^2/2) into
alpha on the host, apply exp(-z^2/2) on the host at the end. Each core then
computes partial[f, b] = sum_n alpha'[n, f] * exp(z.x_n) over its shard.

The device kernel is ScalarE(exp)-bound: 25.7M exps/core at 1.2GHz/128
lanes is a ~167us floor, so the whole design minimizes per-ACT-instruction
overhead (~350ns each: 222-cycle SBUF access bubble + sem/dispatch):
  - exp runs on [128, 2048] / [128, 1536] PSUM tiles (4+3 banks,
    alternating as the two pipeline slots; 8th bank = accumulator), i.e.
    112 ACTIVATEs instead of 196.
  - per b-quarter pass (512 cols), 98 n-tiles stream through the slots as
    "units" alternating top/bottom PE-row halves so paired K=64 cross
    matmuls overlap on the PE array; alpha-weighted acc matmuls land in
    one merged PSUM bank (rows 0:16 / 32:48 via tile_position col 0/32).
  - outputs DMA straight from PSUM to DRAM (no SBUF staging copy).
  - a dummy exp at kernel start pulls the ~1.5us ACT table load off the
    critical chain; first DMAs are split small so the first cross matmul
    starts ASAP.
"""

import sys

if "/opt/trn_rl_repo" not in sys.path:
    sys.path.insert(0, "/opt/trn_rl_repo")

import numpy as np

B = 2048  # batch (queries)
D = 64  # feature dim
F = 16  # output dim
NCORES = 8
N_FULL = 100000
NS = N_FULL // NCORES  # 12500 rows per core
NT = 98  # n-tiles of 128 rows (12544 padded)
NTH = NT // 2  # 49 tiles per partition-half
NSP = NT * 128  # 12544
HALF_COLS = NTH * 128  # 6272
BCH = 512  # b chunk per pass (PSUM acc bank width)
SLOT_UNITS = (4, 3)  # units per alternating PSUM x-slot (4+3 banks)


def _unit_tile(u, nth=NTH):
    """Unit u (0..2*nth-1) -> (dst col-tile index, top-half?)."""
    return u // 2, (u % 2 == 0)


def _pack_core_inputs(z, dataset, alpha):
    """Host-side packing: returns (in_maps, w) where w[b] = exp(-0.5*||z_b||^2)."""
    import ml_dtypes

    z = np.ascontiguousarray(z, dtype=np.float32)
    dataset = np.ascontiguousarray(dataset, dtype=np.float32)
    alpha = np.ascontiguousarray(alpha, dtype=np.float32)

    zT = z.T  # [64, B]
    zt_packed = np.concatenate([zT, zT], axis=0).astype(np.float16)  # [128, B]
    z_sq = np.sum(z.astype(np.float64) ** 2, axis=1)
    w = np.exp(-0.5 * z_sq)  # [B], applied on host at the end

    in_maps = []
    for c in range(NCORES):
        ds_c = dataset[c * NS : (c + 1) * NS]
        al_c = alpha[c * NS : (c + 1) * NS]
        dsp = np.zeros((NSP, D), np.float32)
        dsp[:NS] = ds_c
        alp = np.zeros((NSP, F), np.float32)
        alp[:NS] = al_c
        # fold exp(-x^2/2) into alpha (float64 to keep tiny magnitudes exact)
        xsq = np.sum(dsp.astype(np.float64) ** 2, axis=1)
        alp = (alp.astype(np.float64) * np.exp(-0.5 * xsq)[:, None]).astype(
            np.float32
        )

        dsT = dsp.T  # [64, NSP]
        dst_packed = np.concatenate(
            [dsT[:, :HALF_COLS], dsT[:, HALF_COLS:]], axis=0
        ).astype(np.float16)  # [128, 6272]: tile t rows 0:64, tile NTH+t rows 64:128

        # alpha in device "unit" order: unit u covers tile u//2 (u even,
        # top half) or NTH + u//2 (u odd, bottom half)
        a3 = alp.reshape(NT, 128, F)  # [NT, 128, F]
        order = [u // 2 if u % 2 == 0 else NTH + u // 2 for u in range(NT)]
        alp_packed = np.ascontiguousarray(
            a3[order].transpose(1, 0, 2).reshape(128, NT * F)
        ).astype(ml_dtypes.bfloat16)

        in_maps.append(
            {
                "zt": np.ascontiguousarray(zt_packed),
                "dst": np.ascontiguousarray(dst_packed),
                "alp": alp_packed,
            }
        )
    return in_maps, w


def build_nc(nt=NT):
    """Build the Bass module. nt can be reduced (multiple of 14) for smoke tests."""
    import concourse.bass as bass
    import concourse.tile as tile
    from concourse import bacc, mybir

    assert nt % 14 == 0
    nth = nt // 2
    half_cols = nth * 128

    f32 = mybir.dt.float32
    f16 = mybir.dt.float16
    bf16 = mybir.dt.bfloat16

    nc = bacc.Bacc("TRN2", target_bir_lowering=False, debug=False)
    zt_d = nc.dram_tensor("zt", [128, B], f16, kind="ExternalInput").ap()
    dst_d = nc.dram_tensor("dst", [128, half_cols], f16, kind="ExternalInput").ap()
    alp_d = nc.dram_tensor("alp", [128, nt * F], bf16, kind="ExternalInput").ap()
    out_d = nc.dram_tensor("out", [64, B], f32, kind="ExternalOutput").ap()

    with tile.TileContext(nc) as tc:
        with (
            tc.tile_pool(name="consts", bufs=1) as consts,
            tc.tile_pool(name="g", bufs=2) as gpool,
            tc.tile_pool(name="ps_x", bufs=1, space="PSUM") as ps_x,
            tc.tile_pool(name="ps_acc", bufs=1, space="PSUM") as ps_acc,
        ):
            warm = consts.tile([128, 8], f32, tag="warm", name="warm")
            warm_o = consts.tile([128, 8], bf16, tag="warmo", name="warmo")
            out_sb = consts.tile([64, B], f32, tag="out", name="out_sb")
            zt_sb = consts.tile([128, B], f16, tag="zt", name="zt")
            dst_sb = consts.tile([128, half_cols], f16, tag="dst", name="dst")
            alp_sb = consts.tile([128, nt * F], bf16, tag="alp", name="alp")

            # First-needed pieces first, on the two fast HWDGE queues
            # (sync + scalar). The slow SWDGE gpsimd queue only carries
            # alpha, which isn't needed until the first acc matmul.
            nc.sync.dma_start(out=zt_sb[:, 0:512], in_=zt_d[:, 0:512])
            nc.scalar.dma_start(out=dst_sb[:, 0:256], in_=dst_d[:, 0:256])
            nc.sync.dma_start(out=dst_sb[:, 256:2048], in_=dst_d[:, 256:2048])
            nc.gpsimd.dma_start(out=alp_sb, in_=alp_d)
            nc.sync.dma_start(out=zt_sb[:, 512:B], in_=zt_d[:, 512:B])
            nc.sync.dma_start(
                out=dst_sb[:, 2048:half_cols], in_=dst_d[:, 2048:half_cols]
            )

            # Warm the Exp activation table while the input DMAs fly so
            # the ~1.5us ACT_TABLE_LOAD stays off the critical ACT chain.
            nc.scalar.memzero(warm)
            nc.scalar.activation(
                out=warm_o, in_=warm, func=mybir.ActivationFunctionType.Exp
            )

            # Flat slot list across all 4 b-passes; software-pipelined
            # emission (cross(s), ACT(s-1), acc(s-2)) so the PE queue
            # interleaves next-slot fills ahead of prior-slot drains and
            # the ACT chain never waits on the PE.
            slots = []
            for bq in range(4):
                u = 0
                ci = 0
                while u < nt:
                    nu = SLOT_UNITS[ci % 2]
                    slots.append((bq, ci % 2, u, nu))
                    u += nu
                    ci += 1

            x_t = {}
            g_t = {}
            acc_t = {}

            def emit_cross(s):
                bq, si, u0, nu = slots[s]
                bs = bq * BCH
                x = ps_x.tile([128, nu * BCH], f32, tag=f"x{si}", name=f"x{si}")
                x_t[s] = x
                for j in range(nu):
                    k, top = _unit_tile(u0 + j, nth)
                    rlo = 0 if top else 64
                    nc.tensor.matmul(
                        x[:, j * BCH : (j + 1) * BCH],
                        lhsT=dst_sb[rlo : rlo + 64, k * 128 : (k + 1) * 128],
                        rhs=zt_sb[rlo : rlo + 64, bs : bs + BCH],
                        start=True,
                        stop=True,
                    )

            def emit_act(s):
                _, si, _, nu = slots[s]
                g = gpool.tile([128, nu * BCH], bf16, tag=f"g{si}", name=f"g{si}")
                g_t[s] = g
                nc.scalar.activation(
                    out=g, in_=x_t.pop(s), func=mybir.ActivationFunctionType.Exp
                )

            def emit_acc(s):
                bq, si, u0, nu = slots[s]
                bs = bq * BCH
                if bq not in acc_t:
                    acc_t[bq] = ps_acc.tile([128, BCH], f32, tag="acc", name="acc")
                acc = acc_t[bq]
                g = g_t.pop(s)
                for j in range(nu):
                    uu = u0 + j
                    _, top = _unit_tile(uu, nth)
                    rows = slice(0, F) if top else slice(32, 32 + F)
                    nc.tensor.matmul(
                        acc[rows, :],
                        lhsT=alp_sb[:, uu * F : (uu + 1) * F],
                        rhs=g[:, j * BCH : (j + 1) * BCH],
                        start=(uu < 2),
                        stop=(uu >= nt - 2),
                        tile_position=(0, 0) if top else (0, 32),
                    )
                if u0 + nu == nt:
                    # last slot of this pass: drain acc rows 0:48 in one copy
                    # (rows 16:32 are junk, never read by the host)
                    nc.vector.tensor_copy(
                        out=out_sb[0 : 32 + F, bs : bs + BCH],
                        in_=acc[0 : 32 + F, :],
                    )
                    nc.sync.dma_start(
                        out=out_d[0 : 32 + F, bs : bs + BCH],
                        in_=out_sb[0 : 32 + F, bs : bs + BCH],
                    )

            for s in range(len(slots) + 2):
                if s < len(slots):
                    emit_cross(s)
                if 0 <= s - 1 < len(slots):
                    emit_act(s - 1)
                if 0 <= s - 2 < len(slots):
                    emit_acc(s - 2)

    nc.compile()
    return nc


_NC_CACHE = []


def run_on_cores(in_maps, trace=False, **kwargs):
    from concourse.bass_utils import run_bass_kernel_spmd

    if not _NC_CACHE:
        _NC_CACHE.append(build_nc())
    return run_bass_kernel_spmd(
        _NC_CACHE[0], in_maps, core_ids=list(range(NCORES)), trace=trace, **kwargs
    )


def kernel(z, dataset, alpha):
    in_maps, w = _pack_core_inputs(z, dataset, alpha)
    res = run_on_cores(in_maps, trace=False)
    total = np.zeros((F, B), np.float64)
    for r in res.results:
        o = r["out"].astype(np.float64)  # [64, B]
        total += o[0:F] + o[32 : 32 + F]
    total *= w[None, :]
    return np.ascontiguousarray(total.T.astype(np.float32))


# revision 13
# speedup vs baseline: 1.4934x; 1.0007x over previous
"""RBF kernel regression (Gauss transform) on 8 Trainium2 NeuronCores.

Computes out = K @ alpha where K[b, n] = exp(-||z_b - x_n||^2 / 2),
z: [2048, 64], dataset: [100000, 64], alpha: [100000, 16].

Strategy (sharding_hint): shard dataset/alpha row-wise (N) across 8 cores.
Factorize K = exp(z.x) * exp(-x^2/2) * exp(-z^2/2): fold exp(-x^2/2) into
alpha on the host, apply exp(-z^2/2) on the host at the end. Each core then
computes partial[f, b] = sum_n alpha'[n, f] * exp(z.x_n) over its shard.

The device kernel is ScalarE(exp)-bound: 25.7M exps/core at 1.2GHz/128
lanes is a ~167us floor, so the whole design minimizes per-ACT-instruction
overhead (~350ns each: 222-cycle SBUF access bubble + sem/dispatch):
  - exp runs on [128, 2048] / [128, 1536] PSUM tiles (4+3 banks,
    alternating as the two pipeline slots; 8th bank = accumulator), i.e.
    112 ACTIVATEs instead of 196.
  - per b-quarter pass (512 cols), 98 n-tiles stream through the slots as
    "units" alternating top/bottom PE-row halves so paired K=64 cross
    matmuls overlap on the PE array; alpha-weighted acc matmuls land in
    one merged PSUM bank (rows 0:16 / 32:48 via tile_position col 0/32).
  - per pass the accumulator drains in ONE DVE copy spanning rows 0:48
    (rows 16:32 are junk, never read) to SBUF, then DMAs to DRAM.
  - a dummy exp at kernel start pulls the ~1.5us ACT table load off the
    critical chain; first DMAs are split small, on the two fast HWDGE
    queues, so the first cross matmul starts ASAP.
"""

import sys

if "/opt/trn_rl_repo" not in sys.path:
    sys.path.insert(0, "/opt/trn_rl_repo")

import numpy as np

B = 2048  # batch (queries)
D = 64  # feature dim
F = 16  # output dim
NCORES = 8
N_FULL = 100000
NS = N_FULL // NCORES  # 12500 rows per core
NT = 98  # n-tiles of 128 rows (12544 padded)
NTH = NT // 2  # 49 tiles per partition-half
NSP = NT * 128  # 12544
HALF_COLS = NTH * 128  # 6272
BCH = 512  # b chunk per pass (PSUM acc bank width)
SLOT_UNITS = (4, 3)  # units per alternating PSUM x-slot (4+3 banks)


def _unit_tile(u, nth=NTH):
    """Unit u (0..2*nth-1) -> (dst col-tile index, top-half?)."""
    return u // 2, (u % 2 == 0)


def _pack_core_inputs(z, dataset, alpha):
    """Host-side packing: returns (in_maps, w) where w[b] = exp(-0.5*||z_b||^2)."""
    import ml_dtypes

    z = np.ascontiguousarray(z, dtype=np.float32)
    dataset = np.ascontiguousarray(dataset, dtype=np.float32)
    alpha = np.ascontiguousarray(alpha, dtype=np.float32)

    zT = z.T  # [64, B]
    zt_packed = np.concatenate([zT, zT], axis=0).astype(np.float16)  # [128, B]
    z_sq = np.sum(z.astype(np.float64) ** 2, axis=1)
    w = np.exp(-0.5 * z_sq)  # [B], applied on host at the end

    in_maps = []
    for c in range(NCORES):
        ds_c = dataset[c * NS : (c + 1) * NS]
        al_c = alpha[c * NS : (c + 1) * NS]
        dsp = np.zeros((NSP, D), np.float32)
        dsp[:NS] = ds_c
        alp = np.zeros((NSP, F), np.float32)
        alp[:NS] = al_c
        # fold exp(-x^2/2) into alpha (float64 to keep tiny magnitudes exact)
        xsq = np.sum(dsp.astype(np.float64) ** 2, axis=1)
        alp = (alp.astype(np.float64) * np.exp(-0.5 * xsq)[:, None]).astype(
            np.float32
        )

        dsT = dsp.T  # [64, NSP]
        dst_packed = np.concatenate(
            [dsT[:, :HALF_COLS], dsT[:, HALF_COLS:]], axis=0
        ).astype(np.float16)  # [128, 6272]: tile t rows 0:64, tile NTH+t rows 64:128

        # alpha in device "unit" order: unit u covers tile u//2 (u even,
        # top half) or NTH + u//2 (u odd, bottom half)
        a3 = alp.reshape(NT, 128, F)  # [NT, 128, F]
        order = [u // 2 if u % 2 == 0 else NTH + u // 2 for u in range(NT)]
        alp_packed = np.ascontiguousarray(
            a3[order].transpose(1, 0, 2).reshape(128, NT * F)
        ).astype(ml_dtypes.bfloat16)

        in_maps.append(
            {
                "zt": np.ascontiguousarray(zt_packed),
                "dst": np.ascontiguousarray(dst_packed),
                "alp": alp_packed,
            }
        )
    return in_maps, w


def build_nc(nt=NT):
    """Build the Bass module. nt can be reduced (multiple of 14) for smoke tests."""
    import concourse.bass as bass
    import concourse.tile as tile
    from concourse import bacc, mybir

    assert nt % 14 == 0
    nth = nt // 2
    half_cols = nth * 128

    f32 = mybir.dt.float32
    f16 = mybir.dt.float16
    bf16 = mybir.dt.bfloat16

    nc = bacc.Bacc("TRN2", target_bir_lowering=False, debug=False)
    zt_d = nc.dram_tensor("zt", [128, B], f16, kind="ExternalInput").ap()
    dst_d = nc.dram_tensor("dst", [128, half_cols], f16, kind="ExternalInput").ap()
    alp_d = nc.dram_tensor("alp", [128, nt * F], bf16, kind="ExternalInput").ap()
    out_d = nc.dram_tensor("out", [64, B], f32, kind="ExternalOutput").ap()

    with tile.TileContext(nc) as tc:
        with (
            tc.tile_pool(name="consts", bufs=1) as consts,
            tc.tile_pool(name="g", bufs=3) as gpool,
            tc.tile_pool(name="ps_x", bufs=1, space="PSUM") as ps_x,
            tc.tile_pool(name="ps_acc", bufs=1, space="PSUM") as ps_acc,
        ):
            warm = consts.tile([128, 8], f32, tag="warm", name="warm")
            warm_o = consts.tile([128, 8], bf16, tag="warmo", name="warmo")
            out_sb = consts.tile([64, B], f32, tag="out", name="out_sb")
            zt_sb = consts.tile([128, B], f16, tag="zt", name="zt")
            dst_sb = consts.tile([128, half_cols], f16, tag="dst", name="dst")
            alp_sb = consts.tile([128, nt * F], bf16, tag="alp", name="alp")

            # First-needed pieces first, on the two fast HWDGE queues
            # (sync + scalar). dst is split so early slots' LDWEIGHTS
            # aren't gated on one big transfer; the slow SWDGE gpsimd
            # queue only carries alpha (first needed ~13us in).
            nc.sync.dma_start(out=zt_sb[:, 0:512], in_=zt_d[:, 0:512])
            nc.scalar.dma_start(out=dst_sb[:, 0:256], in_=dst_d[:, 0:256])
            dst_cuts = [256, 512, 1536, 4096, half_cols]
            for a, b in zip(dst_cuts, dst_cuts[1:]):
                a, b = min(a, half_cols), min(b, half_cols)
                if a < b:
                    nc.sync.dma_start(out=dst_sb[:, a:b], in_=dst_d[:, a:b])
            nc.gpsimd.dma_start(out=alp_sb, in_=alp_d)
            nc.sync.dma_start(out=zt_sb[:, 512:B], in_=zt_d[:, 512:B])

            # Warm the Exp activation table while the input DMAs fly so
            # the ~1.5us ACT_TABLE_LOAD stays off the critical ACT chain.
            nc.scalar.memzero(warm)
            nc.scalar.activation(
                out=warm_o, in_=warm, func=mybir.ActivationFunctionType.Exp
            )

            # Flat slot list across all 4 b-passes; software-pipelined
            # emission (cross(s), ACT(s-1), acc(s-2)) so the PE queue
            # interleaves next-slot fills ahead of prior-slot drains and
            # the ACT chain never waits on the PE.
            slots = []
            for bq in range(4):
                u = 0
                ci = 0
                while u < nt:
                    nu = SLOT_UNITS[ci % 2]
                    slots.append((bq, ci % 2, u, nu))
                    u += nu
                    ci += 1

            x_t = {}
            g_t = {}
            acc_t = {}

            def emit_cross(s):
                bq, si, u0, nu = slots[s]
                bs = bq * BCH
                x = ps_x.tile([128, nu * BCH], f32, tag=f"x{si}", name=f"x{si}")
                x_t[s] = x
                for j in range(nu):
                    k, top = _unit_tile(u0 + j, nth)
                    rlo = 0 if top else 64
                    nc.tensor.matmul(
                        x[:, j * BCH : (j + 1) * BCH],
                        lhsT=dst_sb[rlo : rlo + 64, k * 128 : (k + 1) * 128],
                        rhs=zt_sb[rlo : rlo + 64, bs : bs + BCH],
                        start=True,
                        stop=True,
                    )

            def emit_act(s):
                _, si, _, nu = slots[s]
                g = gpool.tile([128, nu * BCH], bf16, tag=f"g{si}", name=f"g{si}")
                g_t[s] = g
                nc.scalar.activation(
                    out=g, in_=x_t.pop(s), func=mybir.ActivationFunctionType.Exp
                )

            def emit_acc(s):
                bq, si, u0, nu = slots[s]
                bs = bq * BCH
                if bq not in acc_t:
                    acc_t[bq] = ps_acc.tile([128, BCH], f32, tag="acc", name="acc")
                acc = acc_t[bq]
                g = g_t.pop(s)
                for j in range(nu):
                    uu = u0 + j
                    _, top = _unit_tile(uu, nth)
                    rows = slice(0, F) if top else slice(32, 32 + F)
                    nc.tensor.matmul(
                        acc[rows, :],
                        lhsT=alp_sb[:, uu * F : (uu + 1) * F],
                        rhs=g[:, j * BCH : (j + 1) * BCH],
                        start=(uu < 2),
                        stop=(uu >= nt - 2),
                        tile_position=(0, 0) if top else (0, 32),
                    )
                if u0 + nu == nt:
                    # last slot of this pass: drain acc rows 0:48 in one copy
                    # (rows 16:32 are junk, never read by the host)
                    nc.vector.tensor_copy(
                        out=out_sb[0 : 32 + F, bs : bs + BCH],
                        in_=acc[0 : 32 + F, :],
                    )
                    nc.sync.dma_start(
                        out=out_d[0 : 32 + F, bs : bs + BCH],
                        in_=out_sb[0 : 32 + F, bs : bs + BCH],
                    )

            for s in range(len(slots) + 2):
                if s < len(slots):
                    emit_cross(s)
                if 0 <= s - 1 < len(slots):
                    emit_act(s - 1)
                if 0 <= s - 2 < len(slots):
                    emit_acc(s - 2)

    nc.compile()
    return nc


_NC_CACHE = []


def run_on_cores(in_maps, trace=False, **kwargs):
    from concourse.bass_utils import run_bass_kernel_spmd

    if not _NC_CACHE:
        _NC_CACHE.append(build_nc())
    return run_bass_kernel_spmd(
        _NC_CACHE[0], in_maps, core_ids=list(range(NCORES)), trace=trace, **kwargs
    )


def kernel(z, dataset, alpha):
    in_maps, w = _pack_core_inputs(z, dataset, alpha)
    res = run_on_cores(in_maps, trace=False)
    total = np.zeros((F, B), np.float64)
    for r in res.results:
        o = r["out"].astype(np.float64)  # [64, B]
        total += o[0:F] + o[32 : 32 + F]
    total *= w[None, :]
    return np.ascontiguousarray(total.T.astype(np.float32))
